# revision 1
# baseline (speedup 1.0000x reference)
"""Conv2d 3x3 VALID (NHWC x HWIO -> NHWC) on 8 Trainium2 NeuronCores.

Strategy: data-parallel over batch (2 images/core). Per core, the conv is an
implicit GEMM over the flattened H*W signal:

    out_flat[co, q] = sum_{r,s,ci} x_flat[ci, q + r*W + s] * w[r, s, ci, co]

with Cout=128 on PSUM partitions and 512-position moving windows (fp16
matmuls, fp32 PSUM accumulate, 1 cycle/row). The 9 taps are packed into five
K<=128 matmuls per window using SBUF-resident copies of the signal shifted by
1 and by W on partitions 64:128, so most matmuls use the full 128-row
contraction. Outputs at flat positions whose
column lands in {W-2, W-1} or row in {H-2, H-1} are garbage and are sliced
away host-side.

Self-contained: hardcodes shapes from the problem spec
  x: (16, 224, 224, 64) f32, w: (3, 3, 64, 128) f32 -> y: (16, 222, 222, 128).
"""
import contextlib
import os
import numpy as np

import concourse.bacc as bacc
import concourse.mybir as mybir
from concourse.tile import TileContext
from concourse.bass_utils import run_bass_kernel_spmd

N_CORES = 8
N_IMG = 2          # images per core
H = W = 224
CIN, COUT = 64, 128
L = H * W          # 50176 flat positions per image
Q = N_IMG * L      # 100352 output positions per core
WIN = 512          # moving-window width (one fp32 PSUM bank)
S = 4096           # slab positions kept in SBUF per iteration
MARGIN = 2 * W + 4
XT_W = Q + WIN     # zero-padded input width

VARIANT = os.environ.get("CONV_VARIANT", "v2")
OUT_DT = os.environ.get("CONV_OUT_DT", "f32")
IN_DT = os.environ.get("CONV_IN_DT", "f16")
A_BUFS = int(os.environ.get("CONV_A_BUFS", "3"))
PS_BUFS = int(os.environ.get("CONV_PS_BUFS", "8"))
O_BUFS = int(os.environ.get("CONV_O_BUFS", "8"))


def make_plan(variant):
    if variant == "v0":
        return [(0, r * W + s, 64, [(r, s), None]) for r in range(3) for s in range(3)]
    if variant == "v1":
        return ([(0, s, 128, [(0, s), (1, s)]) for s in range(3)]
                + [(0, 2 * W + s, 64, [(2, s), None]) for s in range(3)])
    if variant == "v2":
        return ([(0, r * W, 128, [(r, 0), (r, 1)]) for r in range(3)]
                + [(1, 2, 128, [(0, 2), (1, 2)]),
                   (0, 2 * W + 2, 64, [(2, 2), None])])
    raise ValueError(variant)


def build_nc(variant=VARIANT, out_dt=OUT_DT, s_pos=S,
             a_bufs=A_BUFS, ps_bufs=PS_BUFS, o_bufs=O_BUFS, repeat=1,
             order=os.environ.get("CONV_ORDER", "win"), in_dt=IN_DT):
    plan = make_plan(variant)
    n_mm = len(plan)
    ntl = 2 if variant == "v2" else 1
    f32 = mybir.dt.float32
    f32r = {"f32r": mybir.dt.float32r, "f16": mybir.dt.float16,
            "bf16": mybir.dt.bfloat16}[in_dt]
    out_mydt = f32 if out_dt == "f32" else mybir.dt.float16

    nc = bacc.Bacc("TRN2", target_bir_lowering=False, debug=False)
    xt = nc.declare_dram_parameter("xt", [CIN, XT_W], f32r, isOutput=False)
    wt = nc.declare_dram_parameter("wt", [n_mm, 128, COUT], f32r, isOutput=False)
    yt = nc.declare_dram_parameter("yt", [COUT, Q], out_mydt, isOutput=True)

    with TileContext(nc) as tc:
        with (
            tc.tile_pool(name="wpool", bufs=1) as wpool,
            tc.tile_pool(name="apool", bufs=a_bufs) as apool,
            tc.tile_pool(name="opool", bufs=o_bufs) as opool,
            tc.tile_pool(name="pspool", bufs=ps_bufs, space="PSUM") as pspool,
        ):
            w_sb = wpool.tile([128, n_mm * COUT], f32r)
            for i in range(n_mm):
                nc.sync.dma_start(out=w_sb[:, i * COUT:(i + 1) * COUT],
                                  in_=wt[i, :, :])

            n_slabs = (Q + s_pos - 1) // s_pos
            rep = 0
            # repeat>1 wraps the whole body in a HW loop purely for timing:
            # per-pass time = (T(repeat=N) - T(repeat=1)) / (N-1)
            loop_cm = tc.For_i(0, repeat, 1) if repeat > 1 \
                else contextlib.nullcontext()
            with loop_cm:
              for si in range(n_slabs):
                base = si * s_pos
                sh = min(s_pos, Q - base)
                tiles = [apool.tile([128, s_pos + MARGIN], f32r, tag=f"t{t}",
                                    name=f"tile{t}_{rep}_{si}")
                         for t in range(ntl)]
                nc.sync.dma_start(out=tiles[0][0:CIN, 0:sh + MARGIN],
                                  in_=xt[:, base:base + sh + MARGIN])
                if variant == "v1":
                    nc.sync.dma_start(out=tiles[0][CIN:128, 0:sh + 2],
                                      in_=tiles[0][0:CIN, W:W + sh + 2])
                elif variant == "v2":
                    nc.sync.dma_start(out=tiles[0][CIN:128, 0:2 * W + sh],
                                      in_=tiles[0][0:CIN, 1:2 * W + sh + 1])
                    nc.sync.dma_start(out=tiles[1][0:CIN, 0:sh + 2],
                                      in_=tiles[0][0:CIN, 0:sh + 2])
                    nc.sync.dma_start(out=tiles[1][CIN:128, 0:sh + 2],
                                      in_=tiles[0][0:CIN, W:W + sh + 2])

                if order == "win":
                    for q0 in range(0, sh, WIN):
                        acc = pspool.tile([128, WIN], f32)
                        for j, (t, off, kk, _) in enumerate(plan):
                            nc.tensor.matmul(
                                acc[:],
                                w_sb[0:kk, j * COUT:(j + 1) * COUT],
                                tiles[t][0:kk, off + q0: off + q0 + WIN],
                                start=(j == 0),
                                stop=(j == n_mm - 1),
                            )
                        st = opool.tile([128, WIN], out_mydt)
                        nc.vector.tensor_copy(st[:], acc[:])
                        nc.sync.dma_start(out=yt[:, base + q0: base + q0 + WIN],
                                          in_=st[:])
                else:  # tap-major: one weight load serves every window in slab
                    q0s = list(range(0, sh, WIN))
                    accs = [pspool.tile([128, WIN], f32,
                                        name=f"acc_{rep}_{si}_{qi}", tag="acc")
                            for qi in range(len(q0s))]
                    for j, (t, off, kk, _) in enumerate(plan):
                        for qi, q0 in enumerate(q0s):
                            nc.tensor.matmul(
                                accs[qi][:],
                                w_sb[0:kk, j * COUT:(j + 1) * COUT],
                                tiles[t][0:kk, off + q0: off + q0 + WIN],
                                start=(j == 0),
                                stop=(j == n_mm - 1),
                            )
                    for qi, q0 in enumerate(q0s):
                        st = opool.tile([128, WIN], out_mydt)
                        nc.vector.tensor_copy(st[:], accs[qi][:])
                        nc.sync.dma_start(out=yt[:, base + q0: base + q0 + WIN],
                                          in_=st[:])
    nc.compile()
    return nc


def pack_wt(w, variant=VARIANT):
    plan = make_plan(variant)
    wt = np.zeros((len(plan), 128, COUT), dtype=np_in_dt())
    for i, (_, _, _, taps) in enumerate(plan):
        (r0, s0), bot = taps
        wt[i, 0:CIN] = w[r0, s0]
        if bot is not None:
            r1, s1 = bot
            wt[i, CIN:128] = w[r1, s1]
    return wt


def np_in_dt(in_dt=IN_DT):
    if in_dt == "f16":
        return np.float16
    if in_dt == "bf16":
        import ml_dtypes
        return np.dtype(ml_dtypes.bfloat16)
    return np.float32


def prep_xt(xs, in_dt=IN_DT):
    """xs: (N_IMG, H, W, 64) f32 -> (64, XT_W) channel-major flattened + pad."""
    flat = np.ascontiguousarray(xs.transpose(3, 0, 1, 2)).reshape(CIN, N_IMG * L)
    out = np.zeros((CIN, XT_W), dtype=np_in_dt(in_dt))
    out[:, :flat.shape[1]] = flat
    return out


def post_yt(yt_arr):
    """(128, Q) -> (N_IMG, 222, 222, 128) f32."""
    y = np.asarray(yt_arr, dtype=np.float32).reshape(COUT, N_IMG, H, W)
    y = y[:, :, :H - 2, :W - 2]
    return np.ascontiguousarray(y.transpose(1, 2, 3, 0))


_NC_CACHE = {}


def _get_nc():
    key = (VARIANT, OUT_DT, S, A_BUFS, PS_BUFS, O_BUFS)
    if key not in _NC_CACHE:
        _NC_CACHE[key] = build_nc()
    return _NC_CACHE[key]


def make_in_maps(x, w):
    wt = pack_wt(w)
    return [{"xt": prep_xt(x[c * N_IMG:(c + 1) * N_IMG]), "wt": wt}
            for c in range(N_CORES)]


def kernel(x, w):
    x = np.asarray(x, dtype=np.float32)
    w = np.asarray(w, dtype=np.float32)
    nc = _get_nc()
    in_maps = make_in_maps(x, w)
    res = run_bass_kernel_spmd(nc, in_maps, list(range(N_CORES)))
    out = np.empty((N_CORES * N_IMG, H - 2, W - 2, COUT), dtype=np.float32)
    for c in range(N_CORES):
        out[c * N_IMG:(c + 1) * N_IMG] = post_yt(res.results[c]["yt"])
    return out



# revision 2
# speedup vs baseline: 1.8601x; 1.8601x over previous
"""Conv2d 3x3 VALID (NHWC x HWIO -> NHWC) on 8 Trainium2 NeuronCores.

Strategy ("dual"): data-parallel over batch (2 images/core), and within a
core the two images run concurrently on the two 64-row tiles of the PE
array (64x128 row-tiling mode):

  - SBUF partitions 0:64  hold image 0's 64 channels (flat H*W signal)
  - SBUF partitions 64:128 hold image 1's 64 channels
  - PE tile T0 (rows 0:64)  computes image 0, tile T8 (rows 64:128) image 1

Per 512-position window each image needs 9 K=64 matmuls (one per conv tap,
each just a different column offset into the same SBUF slab - no shifted
data copies), accumulated in that image's own PSUM bank.  The two tiles run
concurrently, so the effective cost is 9 x 512 cycles per *two* windows =
4.5 K=128-equivalent matmuls per window (the dense-packing floor).

Outputs at flat positions whose column lands in {W-2, W-1} or row in
{H-2, H-1} are garbage and are sliced away host-side.

Self-contained: hardcodes shapes from the problem spec
  x: (16, 224, 224, 64) f32, w: (3, 3, 64, 128) f32 -> y: (16, 222, 222, 128).
"""
import contextlib
import os
import numpy as np

import concourse.bacc as bacc
import concourse.mybir as mybir
from concourse.tile import TileContext
from concourse.bass_utils import run_bass_kernel_spmd

N_CORES = 8
N_IMG = 2          # images per core
H = W = 224
CIN, COUT = 64, 128
L = H * W          # 50176 flat positions per image
WIN = 512          # window width (one fp32 PSUM bank)
MARGIN = 2 * W + 4
XT2_W = L + WIN    # zero-padded per-image input width

VARIANT = os.environ.get("CONV_VARIANT", "dual")
OUT_DT = os.environ.get("CONV_OUT_DT", "f16")
IN_DT = os.environ.get("CONV_IN_DT", "f16")
S = int(os.environ.get("CONV_S", "7168"))        # slab positions (dual)
WPAIR = int(os.environ.get("CONV_WPAIR", "2"))   # windows per tap-group
A_BUFS = int(os.environ.get("CONV_A_BUFS", "3"))
PS_BUFS = int(os.environ.get("CONV_PS_BUFS", "8"))
O_BUFS = int(os.environ.get("CONV_O_BUFS", "3"))
OGRAN = int(os.environ.get("CONV_OGRAN", "14"))  # windows per output DMA
EVAC = os.environ.get("CONV_EVAC", "vs")         # v, s, or vs (alternate)

TAPS = [(r, s) for r in range(3) for s in range(3)]


def np_in_dt(in_dt=None):
    in_dt = in_dt or IN_DT
    if in_dt == "f16":
        return np.float16
    if in_dt == "bf16":
        import ml_dtypes
        return np.dtype(ml_dtypes.bfloat16)
    return np.float32


def my_in_dt(in_dt=None):
    in_dt = in_dt or IN_DT
    return {"f32r": mybir.dt.float32r, "f16": mybir.dt.float16,
            "bf16": mybir.dt.bfloat16}[in_dt]


# ---------------------------------------------------------------- dual ----

def build_dual(out_dt=OUT_DT, s_pos=None, a_bufs=A_BUFS, ps_bufs=PS_BUFS,
               o_bufs=O_BUFS, repeat=1, in_dt=IN_DT, wpair=None,
               ogran=None, evac=None):
    s_pos = s_pos or S
    wpair = wpair or WPAIR
    ogran = ogran or OGRAN
    evac = evac or EVAC
    f32 = mybir.dt.float32
    idt = my_in_dt(in_dt)
    out_mydt = f32 if out_dt == "f32" else mybir.dt.float16

    assert L % s_pos == 0 and s_pos % WIN == 0
    n_slabs = L // s_pos
    n_win = s_pos // WIN           # windows per slab (per image)
    assert n_win % wpair == 0 and n_win % ogran == 0

    nc = bacc.Bacc("TRN2", target_bir_lowering=False, debug=False)
    # rows 0:64 image0 channels, rows 64:128 image1 channels
    xt2 = nc.declare_dram_parameter("xt2", [128, XT2_W], idt, isOutput=False)
    # tap j weights replicated on both partition halves
    wt2 = nc.declare_dram_parameter("wt2", [128, 9 * COUT], idt, isOutput=False)
    # cols 0:L image0, cols L:2L image1
    yt = nc.declare_dram_parameter("yt", [COUT, 2 * L], out_mydt, isOutput=True)

    with TileContext(nc) as tc:
        with (
            tc.tile_pool(name="wpool", bufs=1) as wpool,
            tc.tile_pool(name="apool", bufs=a_bufs) as apool,
            tc.tile_pool(name="opool", bufs=o_bufs) as opool,
            tc.tile_pool(name="pspool", bufs=ps_bufs, space="PSUM") as pspool,
        ):
            w_sb = wpool.tile([128, 9 * COUT], idt)
            nc.sync.dma_start(out=w_sb[:, :], in_=wt2[:, :])

            loop_cm = tc.For_i(0, repeat, 1) if repeat > 1 \
                else contextlib.nullcontext()
            ev = 0
            with loop_cm:
              for si in range(n_slabs):
                base = si * s_pos
                xd = apool.tile([128, s_pos + MARGIN], idt, tag="xd",
                                name=f"xd_{si}")
                nc.sync.dma_start(out=xd[:, :],
                                  in_=xt2[:, base:base + s_pos + MARGIN])
                for og in range(0, n_win, ogran):
                    sts = [opool.tile([128, ogran * WIN], out_mydt,
                                      tag=f"st{t}", name=f"st{t}_{si}_{og}")
                           for t in range(2)]
                    for wp in range(og, og + ogran, wpair):
                        accs = [[pspool.tile([128, WIN], f32, tag="acc",
                                             name=f"acc_{si}_{t}_{wi}")
                                 for wi in range(wp, wp + wpair)]
                                for t in range(2)]
                        for j in range(9):
                            r, s = TAPS[j]
                            off = r * W + s
                            st_j = (j == 0)
                            sp_j = (j == 8)
                            for t in range(2):
                                p0 = t * 64
                                for wi in range(wpair):
                                    q0 = (wp + wi) * WIN + off
                                    nc.tensor.matmul(
                                        accs[t][wi][:],
                                        w_sb[p0:p0 + 64,
                                             j * COUT:(j + 1) * COUT],
                                        xd[p0:p0 + 64, q0:q0 + WIN],
                                        start=st_j, stop=sp_j,
                                    )
                        # evacuate PSUM -> SBUF (cast), alternating engines
                        for t in range(2):
                            for wi in range(wpair):
                                c0 = (wp - og + wi) * WIN
                                dst = sts[t][:, c0:c0 + WIN]
                                src = accs[t][wi][:]
                                if evac == "v" or (evac == "vs" and ev % 2 == 0):
                                    nc.vector.tensor_copy(dst, src)
                                else:
                                    nc.scalar.activation(
                                        dst, src,
                                        mybir.ActivationFunctionType.Copy)
                                ev += 1
                    for t in range(2):
                        col = t * L + base + og * WIN
                        nc.sync.dma_start(
                            out=yt[:, col:col + ogran * WIN],
                            in_=sts[t][:, :])
    nc.compile()
    return nc


def prep_xt2(xs, in_dt=None):
    """xs: (2, H, W, 64) f32 -> (128, XT2_W): img0 chans on rows 0:64."""
    out = np.zeros((128, XT2_W), dtype=np_in_dt(in_dt))
    for t in range(N_IMG):
        flat = np.ascontiguousarray(xs[t].transpose(2, 0, 1)).reshape(CIN, L)
        out[t * CIN:(t + 1) * CIN, :L] = flat
    return out


def pack_wt2(w, in_dt=None):
    """w: (3,3,64,128) -> (128, 9*128), tap j on both partition halves."""
    wt = np.zeros((128, 9 * COUT), dtype=np_in_dt(in_dt))
    for j, (r, s) in enumerate(TAPS):
        wt[0:CIN, j * COUT:(j + 1) * COUT] = w[r, s]
        wt[CIN:128, j * COUT:(j + 1) * COUT] = w[r, s]
    return wt


def post_yt_dual(yt_arr):
    """(128, 2L) -> (2, 222, 222, 128) f32."""
    y = np.asarray(yt_arr, dtype=np.float32).reshape(COUT, N_IMG, H, W)
    y = y[:, :, :H - 2, :W - 2]
    return np.ascontiguousarray(y.transpose(1, 2, 3, 0))


# ------------------------------------------------------------ v2 (old) ----

V2_S = 4096
V2_XT_W = N_IMG * L + WIN


def make_plan_v2():
    return ([(0, r * W, 128, [(r, 0), (r, 1)]) for r in range(3)]
            + [(1, 2, 128, [(0, 2), (1, 2)]),
               (0, 2 * W + 2, 64, [(2, 2), None])])


def build_v2(out_dt=OUT_DT, s_pos=V2_S, a_bufs=A_BUFS, ps_bufs=PS_BUFS,
             o_bufs=8, repeat=1, in_dt=IN_DT):
    plan = make_plan_v2()
    n_mm = len(plan)
    Q = N_IMG * L
    f32 = mybir.dt.float32
    idt = my_in_dt(in_dt)
    out_mydt = f32 if out_dt == "f32" else mybir.dt.float16

    nc = bacc.Bacc("TRN2", target_bir_lowering=False, debug=False)
    xt = nc.declare_dram_parameter("xt", [CIN, V2_XT_W], idt, isOutput=False)
    wt = nc.declare_dram_parameter("wt", [n_mm, 128, COUT], idt, isOutput=False)
    yt = nc.declare_dram_parameter("yt", [COUT, Q], out_mydt, isOutput=True)

    with TileContext(nc) as tc:
        with (
            tc.tile_pool(name="wpool", bufs=1) as wpool,
            tc.tile_pool(name="apool", bufs=a_bufs) as apool,
            tc.tile_pool(name="opool", bufs=o_bufs) as opool,
            tc.tile_pool(name="pspool", bufs=ps_bufs, space="PSUM") as pspool,
        ):
            w_sb = wpool.tile([128, n_mm * COUT], idt)
            for i in range(n_mm):
                nc.sync.dma_start(out=w_sb[:, i * COUT:(i + 1) * COUT],
                                  in_=wt[i, :, :])

            n_slabs = (Q + s_pos - 1) // s_pos
            loop_cm = tc.For_i(0, repeat, 1) if repeat > 1 \
                else contextlib.nullcontext()
            with loop_cm:
              for si in range(n_slabs):
                base = si * s_pos
                sh = min(s_pos, Q - base)
                tiles = [apool.tile([128, s_pos + MARGIN], idt, tag=f"t{t}",
                                    name=f"tile{t}_{si}")
                         for t in range(2)]
                # all four halves straight from HBM (no SBUF->SBUF chains)
                nc.sync.dma_start(out=tiles[0][0:CIN, 0:sh + MARGIN],
                                  in_=xt[:, base:base + sh + MARGIN])
                nc.sync.dma_start(out=tiles[0][CIN:128, 0:2 * W + sh],
                                  in_=xt[:, base + 1:base + 1 + 2 * W + sh])
                nc.sync.dma_start(out=tiles[1][0:CIN, 0:sh + 2],
                                  in_=xt[:, base:base + sh + 2])
                nc.sync.dma_start(out=tiles[1][CIN:128, 0:sh + 2],
                                  in_=xt[:, base + W:base + W + sh + 2])

                for q0 in range(0, sh, WIN):
                    acc = pspool.tile([128, WIN], f32)
                    for j, (t, off, kk, _) in enumerate(plan):
                        nc.tensor.matmul(
                            acc[:],
                            w_sb[0:kk, j * COUT:(j + 1) * COUT],
                            tiles[t][0:kk, off + q0: off + q0 + WIN],
                            start=(j == 0),
                            stop=(j == n_mm - 1),
                        )
                    st = opool.tile([128, WIN], out_mydt)
                    nc.vector.tensor_copy(st[:], acc[:])
                    nc.sync.dma_start(out=yt[:, base + q0: base + q0 + WIN],
                                      in_=st[:])
    nc.compile()
    return nc


def pack_wt_v2(w, in_dt=None):
    plan = make_plan_v2()
    wt = np.zeros((len(plan), 128, COUT), dtype=np_in_dt(in_dt))
    for i, (_, _, _, taps) in enumerate(plan):
        (r0, s0), bot = taps
        wt[i, 0:CIN] = w[r0, s0]
        if bot is not None:
            r1, s1 = bot
            wt[i, CIN:128] = w[r1, s1]
    return wt


def prep_xt_v2(xs, in_dt=None):
    flat = np.ascontiguousarray(xs.transpose(3, 0, 1, 2)).reshape(CIN, N_IMG * L)
    out = np.zeros((CIN, V2_XT_W), dtype=np_in_dt(in_dt))
    out[:, :flat.shape[1]] = flat
    return out


def post_yt_v2(yt_arr):
    y = np.asarray(yt_arr, dtype=np.float32).reshape(COUT, N_IMG, H, W)
    y = y[:, :, :H - 2, :W - 2]
    return np.ascontiguousarray(y.transpose(1, 2, 3, 0))


# ------------------------------------------------------------ driver ------

def build_nc(repeat=1, variant=None):
    variant = variant or VARIANT
    if variant == "dual":
        return build_dual(repeat=repeat)
    return build_v2(repeat=repeat)


_NC_CACHE = {}


def _get_nc():
    key = (VARIANT, OUT_DT, IN_DT, S, WPAIR, A_BUFS, PS_BUFS, O_BUFS, OGRAN,
           EVAC)
    if key not in _NC_CACHE:
        _NC_CACHE[key] = build_nc()
    return _NC_CACHE[key]


def make_in_maps(x, w, variant=None):
    variant = variant or VARIANT
    if variant == "dual":
        wt = pack_wt2(w)
        return [{"xt2": prep_xt2(x[c * N_IMG:(c + 1) * N_IMG]), "wt2": wt}
                for c in range(N_CORES)]
    wt = pack_wt_v2(w)
    return [{"xt": prep_xt_v2(x[c * N_IMG:(c + 1) * N_IMG]), "wt": wt}
            for c in range(N_CORES)]


def kernel(x, w):
    x = np.asarray(x, dtype=np.float32)
    w = np.asarray(w, dtype=np.float32)
    nc = _get_nc()
    in_maps = make_in_maps(x, w)
    res = run_bass_kernel_spmd(nc, in_maps, list(range(N_CORES)))
    out = np.empty((N_CORES * N_IMG, H - 2, W - 2, COUT), dtype=np.float32)
    post = post_yt_dual if VARIANT == "dual" else post_yt_v2
    for c in range(N_CORES):
        out[c * N_IMG:(c + 1) * N_IMG] = post(res.results[c]["yt"])
    return out
